# revision 22
# baseline (speedup 1.0000x reference)
"""Trainium2 Bass kernel for nn_Projector: rotate volume + trilinear sample + sum.

Strategy: data-parallel over the 16 rotations (2 per NeuronCore). Each core is
shipped only a 1/8 z-shard of a zero-padded f16 volume (575KB) plus tiny
per-rotation coordinate tables; the full padded volume is reassembled on-device
with an AllGather over NeuronLink. Per k-plane, per-sample voxel coordinates /
trilinear weights are computed with wide-tile DVE ops, the 8 trilinear corners
are fetched as four plane-wide indirect-DMA pair-gathers (contiguous (x0,x0+1)
runs, dz/dy shifts via element_offset), and the lerp tree + k-accumulation run
on DVE. Exact float32 grid_sample semantics (align_corners=True, zeros padding)
via clamping into the zero shell of the padded volume.
"""

import sys

sys.path.insert(0, "/opt/trn_rl_repo")

import numpy as np

import concourse.bass as bass
import concourse.mybir as mybir
from concourse.tile import TileContext
from concourse.bass_utils import run_bass_kernel_spmd

from concourse import mybir as _mybir
from concourse import tile as _tile
from concourse.vector_clock import ScopedClock as _ScopedClock


def _patched_drain_and_barrier(self, tick_clock, wait_clock):
    nc = self.nc
    carrier = nc.sync.nop(nofuse=True)
    wait_clock.add_sem_waits(carrier.ins, _ScopedClock({None: tick_clock.global_clock}))
    si = carrier.ins.sync_info
    waits = list(si.on_wait) if si is not None else []
    if len(waits) > 1:
        carrier.ins.sync_info = _mybir.SyncInfo(on_wait=waits[:1], on_update=list(si.on_update))
        for w in waits[1:]:
            extra = nc.sync.nop(nofuse=True)
            extra.ins.sync_info = _mybir.SyncInfo(on_wait=[w], on_update=[])
    nc.sync.drain()

    nc.all_engine_barrier()
    assert self.sems is not None
    popped = nc._tile_sem_poison_stack.pop()
    assert popped is self._sem_poison
    nc.clear_and_free_semaphores(list(self.sems.allocated().values()))
    nc.all_engine_barrier()


_orig_add_instruction = _tile.TileContext._add_instruction
_nop_counter = [0]


def _patched_add_instruction(self, inst):
    si = getattr(inst, "sync_info", None)
    if si is not None and si.on_wait is not None and len(si.on_wait) > 1:
        waits = list(si.on_wait)
        for w in waits[:-1]:
            _nop_counter[0] += 1
            nop = _mybir.InstNoOp(
                name=f"{inst.name}-mw{_nop_counter[0]}",
                engine=inst.engine,
                bass_nofuse=True,
                sync_info=_mybir.SyncInfo(on_wait=[w], on_update=[]),
            )
            _orig_add_instruction(self, nop)
        inst.sync_info = _mybir.SyncInfo(
            on_wait=waits[-1:], on_update=list(si.on_update)
        )
    _orig_add_instruction(self, inst)


def apply():
    _tile.TileContext._drain_and_barrier = _patched_drain_and_barrier
    _tile.TileContext._add_instruction = _patched_add_instruction

apply()

# --- persistent NEFF compile cache -----------------------------------------
# The bass_exec compile path (bass2jax.neuronx_cc_hook) bypasses libneuronxla's
# on-disk cache, so every fresh process pays the full walrus compile (~90s for
# this kernel). Memoize the hook on the serialized HLO bytes.
import hashlib as _hashlib
import os as _os

from concourse import bass2jax as _bass2jax

_CC_CACHE_DIR = _os.path.expanduser("~/.neuron-compile-cache-bass")
_orig_cc_hook = _bass2jax.neuronx_cc_hook


def _cached_cc_hook(code, code_format, platform_version, file_prefix):
    try:
        is_bass = b"bass_exec" in code
    except TypeError:
        is_bass = False
    if not is_bass:
        return _orig_cc_hook(code, code_format, platform_version, file_prefix)
    key = _hashlib.sha256(bytes(code)).hexdigest()
    path = _os.path.join(_CC_CACHE_DIR, key + ".neffcc")
    try:
        with open(path, "rb") as f:
            return 0, f.read()
    except OSError:
        pass
    ret = _orig_cc_hook(code, code_format, platform_version, file_prefix)
    try:
        status, data = ret
        if status == 0 and isinstance(data, (bytes, bytearray)):
            _os.makedirs(_CC_CACHE_DIR, exist_ok=True)
            tmp = f"{path}.tmp{_os.getpid()}"
            with open(tmp, "wb") as f:
                f.write(data)
            _os.replace(tmp, path)
    except Exception:
        pass
    return ret


_bass2jax.neuronx_cc_hook = _cached_cc_hook
# ---------------------------------------------------------------------------

S = 128
B = 16
N_CORES = 8
B_PER_CORE = B // N_CORES
PV = 132            # padded per-axis extent: index range [-2, 129] stored at +2
PV2 = PV * PV
NFLAT = PV * PV * PV
SHARD_ROWS = PV2 // N_CORES  # 2178 rows of [PV2, PV] layout per core
NCOPY = NFLAT - (PV2 + PV + 1)  # oct rows actually backed by v2 data
ALU = mybir.AluOpType
F32 = mybir.dt.float32
F16 = mybir.dt.float16
I32 = mybir.dt.int32

_nc_cache = {}
_last_exec_ns = 0
_chunk_walls = []


def _build_bass():
    nc = bass.Bass(num_devices=N_CORES)
    vsh_in = nc.declare_dram_parameter("vsh", [SHARD_ROWS, PV], F16, isOutput=False)
    tabs_in = nc.declare_dram_parameter("tabs", [S, 18], F32, isOutput=False)
    out_e = nc.declare_dram_parameter("out", [B_PER_CORE * S, S], F16, isOutput=True)

    # --- reassemble the full padded volume on-device (pre-TileContext, with
    # explicit completion semaphores: indirect-DMA reads of the oct table are
    # not dependency-tracked against these async writes) ---
    bounce = nc.dram_tensor("bounce", [SHARD_ROWS, PV], F16)
    v2 = nc.dram_tensor("v2", [NFLAT, 1], F16)
    oct_t = nc.dram_tensor("oct", [NFLAT, 8], F16)
    s0 = nc.alloc_semaphore("v2_ready")
    nc.gpsimd.dma_start(bounce[:], vsh_in[:]).then_inc(s0, 16)
    nc.gpsimd.wait_ge(s0, 16)
    nc.gpsimd.collective_compute(
        "AllGather",
        ALU.bypass,
        replica_groups=[list(range(N_CORES))],
        ins=[bounce[:].opt()],
        outs=[v2[:].opt()],
    ).then_inc(s0, 1)
    nc.sync.wait_ge(s0, 17)
    # flat-shifted oct table: oct[f, c] = v2[f + dz*PV2 + dy*PV + dx],
    # c = dz*4 + dy*2 + dx. One gathered 16B row -> all 8 trilinear corners.
    CH = 65535  # AP dim counts are 16-bit ISA fields; chunk the big copies
    n_copy_dmas = 0
    with nc.allow_non_contiguous_dma(reason="strided oct-table interleave build"):
        for c in range(8):
            dz, dy, dx = (c >> 2) & 1, (c >> 1) & 1, c & 1
            off = dz * PV2 + dy * PV + dx
            for lo in range(0, NCOPY, CH):
                hi = min(lo + CH, NCOPY)
                nc.sync.dma_start(
                    out=oct_t[lo:hi, c : c + 1],
                    in_=v2[off + lo : off + hi, :],
                ).then_inc(s0, 16)
                n_copy_dmas += 1
    nc.gpsimd.wait_ge(s0, 17 + n_copy_dmas * 16)

    with TileContext(nc) as tc:
        with (
            tc.tile_pool(name="const", bufs=1) as cpool,
            tc.tile_pool(name="acc", bufs=1) as apool,
            tc.tile_pool(name="work", bufs=3) as wpool,
        ):
            # --- constants ---
            tabs = cpool.tile([S, 18], F32, tag="tabs")
            nc.sync.dma_start(out=tabs[:], in_=tabs_in[:])
            fpl_i = cpool.tile([S, S], I32, tag="fpl_i")
            nc.gpsimd.iota(fpl_i[:], pattern=[[1, S]], base=0, channel_multiplier=0)
            fpl = cpool.tile([S, S], F32, tag="fpl")
            nc.vector.tensor_copy(out=fpl[:], in_=fpl_i[:])

            for b in range(B_PER_CORE):
                co = b * 9  # column offset in tabs: [pu(3), v(3), w(3)]
                # per-axis planes, axes concatenated on the free dim
                jv = cpool.tile([S, 3 * S], F32, tag=f"jv{b}")
                basep = cpool.tile([S, 3 * S], F32, tag=f"basep{b}")
                winc = cpool.tile([S, 3 * S], F32, tag=f"winc{b}")
                for a in range(3):
                    blk = slice(a * S, (a + 1) * S)
                    nc.vector.tensor_scalar(
                        out=jv[:, blk], in0=fpl[:],
                        scalar1=tabs[:, co + 3 + a : co + 4 + a], scalar2=None,
                        op0=ALU.mult,
                    )
                    nc.vector.tensor_scalar(
                        out=basep[:, blk], in0=fpl[:],
                        scalar1=0.0, scalar2=tabs[:, co + a : co + 1 + a],
                        op0=ALU.mult, op1=ALU.add,
                    )
                    nc.vector.tensor_scalar(
                        out=winc[:, blk], in0=fpl[:],
                        scalar1=0.0, scalar2=tabs[:, co + 6 + a : co + 7 + a],
                        op0=ALU.mult, op1=ALU.add,
                    )
                w32 = cpool.tile([S, 3 * S], F32, tag=f"w32{b}")
                nc.vector.tensor_scalar(
                    out=w32[:], in0=winc[:], scalar1=32.0, scalar2=None,
                    op0=ALU.mult,
                )

                acc = apool.tile([S, S], F32, tag=f"acc{b}")
                nc.vector.memset(acc[:], 0.0)
                cur = apool.tile([S, 3 * S], F32, tag=f"cur{b}")

                def advance(k):
                    # set cur to plane k's coordinates
                    if k % 32 == 0:
                        # exact re-sync every 32 planes: caps f32 += drift
                        if k > 0:
                            nc.vector.tensor_tensor(
                                out=basep[:], in0=basep[:], in1=w32[:], op=ALU.add
                            )
                        nc.vector.tensor_tensor(
                            out=cur[:], in0=jv[:], in1=basep[:], op=ALU.add
                        )
                    else:
                        nc.vector.tensor_tensor(
                            out=cur[:], in0=cur[:], in1=winc[:], op=ALU.add
                        )

                def coords_and_gather():
                    # coordinates + index + gather for one k-plane
                    fr = wpool.tile([S, 3 * S], F32, tag="fr")
                    idx = wpool.tile([S, S], I32, tag="idx")
                    vball = wpool.tile([S, S * 8], F16, tag="vball")
                    sc = wpool.tile([S, 3 * S], F32, tag="sc")
                    nc.vector.tensor_scalar(
                        out=sc[:], in0=cur[:], scalar1=-1.0, scalar2=128.0,
                        op0=ALU.max, op1=ALU.min,
                    )
                    i0 = wpool.tile([S, 3 * S], I32, tag="i0")
                    nc.vector.tensor_scalar(
                        out=i0[:], in0=sc[:], scalar1=0.5, scalar2=None,
                        op0=ALU.subtract,
                    )
                    ff = wpool.tile([S, 3 * S], F32, tag="ff")
                    nc.vector.tensor_copy(out=ff[:], in_=i0[:])
                    nc.vector.tensor_tensor(
                        out=fr[:], in0=sc[:], in1=ff[:], op=ALU.subtract
                    )
                    # flat index = (z0+2)*PV2 + (y0+2)*PV + (x0+2)
                    t1 = wpool.tile([S, S], F32, tag="t1")
                    nc.vector.scalar_tensor_tensor(
                        out=t1[:], in0=ff[:, S : 2 * S], scalar=float(PV),
                        in1=ff[:, 0:S], op0=ALU.mult, op1=ALU.add,
                    )
                    t2 = wpool.tile([S, S], F32, tag="t2")
                    nc.vector.scalar_tensor_tensor(
                        out=t2[:], in0=ff[:, 2 * S : 3 * S], scalar=float(PV2),
                        in1=t1[:], op0=ALU.mult, op1=ALU.add,
                    )
                    nc.vector.tensor_scalar(
                        out=idx[:], in0=t2[:],
                        scalar1=float(2 * PV2 + 2 * PV + 2), scalar2=None,
                        op0=ALU.add,
                    )
                    # gather: one 16B oct row (8 corners) per sample, one
                    # offset per partition per call -> 128 calls per plane
                    for j in range(S):
                        nc.gpsimd.indirect_dma_start(
                            out=vball[:, j * 8 : (j + 1) * 8],
                            out_offset=None,
                            in_=oct_t[:],
                            in_offset=bass.IndirectOffsetOnAxis(
                                ap=idx[:, j : j + 1], axis=0
                            ),
                        )
                    return fr, vball

                def lerp_acc(fr, vball):
                    vc = wpool.tile([S, S * 8], F32, tag="vc")
                    nc.vector.tensor_copy(out=vc[:], in_=vball[:])
                    v3 = vc[:].rearrange("p (j c) -> p j c", c=8)
                    # x lerp: 4 pairs per sample
                    frx = fr[:, 0:S].rearrange("p (j o) -> p j o", o=1).broadcast_to(
                        [S, S, 4]
                    )
                    xd = wpool.tile([S, S * 4], F32, tag="xd")
                    xd3 = xd[:].rearrange("p (j c) -> p j c", c=4)
                    nc.vector.tensor_tensor(
                        out=xd3, in0=v3[:, :, 1::2], in1=v3[:, :, 0::2],
                        op=ALU.subtract,
                    )
                    xm = wpool.tile([S, S * 4], F32, tag="xm")
                    xm3 = xm[:].rearrange("p (j c) -> p j c", c=4)
                    nc.vector.tensor_tensor(out=xm3, in0=xd3, in1=frx, op=ALU.mult)
                    xl = wpool.tile([S, S * 4], F32, tag="xl")
                    xl3 = xl[:].rearrange("p (j c) -> p j c", c=4)
                    nc.vector.tensor_tensor(
                        out=xl3, in0=v3[:, :, 0::2], in1=xm3, op=ALU.add
                    )
                    # y lerp: 2 pairs
                    fry = fr[:, S : 2 * S].rearrange(
                        "p (j o) -> p j o", o=1
                    ).broadcast_to([S, S, 2])
                    yd = wpool.tile([S, S * 2], F32, tag="yd")
                    yd3 = yd[:].rearrange("p (j c) -> p j c", c=2)
                    nc.vector.tensor_tensor(
                        out=yd3, in0=xl3[:, :, 1::2], in1=xl3[:, :, 0::2],
                        op=ALU.subtract,
                    )
                    ym = wpool.tile([S, S * 2], F32, tag="ym")
                    ym3 = ym[:].rearrange("p (j c) -> p j c", c=2)
                    nc.vector.tensor_tensor(out=ym3, in0=yd3, in1=fry, op=ALU.mult)
                    yl = wpool.tile([S, S * 2], F32, tag="yl")
                    yl3 = yl[:].rearrange("p (j c) -> p j c", c=2)
                    nc.vector.tensor_tensor(
                        out=yl3, in0=xl3[:, :, 0::2], in1=ym3, op=ALU.add
                    )
                    # z lerp + accumulate
                    zd = wpool.tile([S, S], F32, tag="zd")
                    nc.vector.tensor_tensor(
                        out=zd[:], in0=yl3[:, :, 1], in1=yl3[:, :, 0],
                        op=ALU.subtract,
                    )
                    zm = wpool.tile([S, S], F32, tag="zm")
                    nc.vector.tensor_tensor(
                        out=zm[:], in0=zd[:], in1=fr[:, 2 * S : 3 * S], op=ALU.mult
                    )
                    zs = wpool.tile([S, S], F32, tag="zs")
                    nc.vector.tensor_tensor(
                        out=zs[:], in0=yl3[:, :, 0], in1=zm[:], op=ALU.add
                    )
                    nc.vector.tensor_tensor(
                        out=acc[:], in0=acc[:], in1=zs[:], op=ALU.add
                    )

                # software pipeline: gather(k+1) issued before lerp(k) so the
                # indirect-DMA stream stays busy under the DVE lerp tree
                advance(0)
                pend = coords_and_gather()
                for k in range(S):
                    if k < S - 1:
                        advance(k + 1)
                        nxt = coords_and_gather()
                    lerp_acc(*pend)
                    if k < S - 1:
                        pend = nxt

                out16 = apool.tile([S, S], F16, tag=f"out16_{b}")
                nc.vector.tensor_copy(out=out16[:], in_=acc[:])
                nc.sync.dma_start(out=out_e[b * S : (b + 1) * S, :], in_=out16[:])
    return nc


def kernel(rotmat, vol, proj_axis):
    rotmat = np.asarray(rotmat, dtype=np.float32)
    vol = np.asarray(vol, dtype=np.float32)
    pa = int(np.asarray(proj_axis))
    assert rotmat.shape == (B, 3, 3) and vol.shape == (S, S, S)
    assert pa in (1, 2, 3), f"proj_axis={pa} unsupported"

    # host-built zero-padded f16 volume: V2[z+2, y+2, x+2] = vol[z, y, x]
    v2 = np.zeros((PV, PV, PV), dtype=np.float16)
    v2[2 : 2 + S, 2 : 2 + S, 2 : 2 + S] = vol
    v2_rows = v2.reshape(PV2, PV)

    p_idx = np.arange(S, dtype=np.float64)

    in_maps = []
    for core in range(N_CORES):
        tabs = np.empty((S, 18), dtype=np.float32)
        for bl in range(B_PER_CORE):
            R = rotmat[core * B_PER_CORE + bl].astype(np.float64)
            # lattice directions: i -> R[1], j -> R[0], k -> R[2]
            dirs = [R[1], R[0], R[2]]
            wdir = dirs.pop(pa - 1)
            u, v = dirs  # output row (partition) dir, output col dir
            co = bl * 9
            for a in range(3):
                c0 = 63.5 - 63.5 * (u[a] + v[a] + wdir[a])
                tabs[:, co + a] = (c0 + p_idx * u[a]).astype(np.float32)
                tabs[:, co + 3 + a] = np.float32(v[a])
                tabs[:, co + 6 + a] = np.float32(wdir[a])
        in_maps.append(
            {
                "vsh": v2_rows[core * SHARD_ROWS : (core + 1) * SHARD_ROWS],
                "tabs": tabs,
            }
        )

    key = "nc"
    if key not in _nc_cache:
        _nc_cache[key] = _build_bass()
    nc = _nc_cache[key]

    global _last_exec_ns, _chunk_walls
    _last_exec_ns = 0
    _chunk_walls = []
    import os as _os, time as _time
    _trace = _os.environ.get("BASS_PROJ_TRACE") == "1"
    _t0 = _time.time()
    try:
        res = run_bass_kernel_spmd(
            nc, in_maps, core_ids=list(range(N_CORES)), trace=_trace
        )
    except ModuleNotFoundError:
        res = run_bass_kernel_spmd(nc, in_maps, core_ids=list(range(N_CORES)))
    _chunk_walls.append(_time.time() - _t0)
    if res.exec_time_ns:
        _last_exec_ns += res.exec_time_ns

    total = np.empty((B, S, S), dtype=np.float32)
    for c in range(N_CORES):
        o = res.results[c]["out"].astype(np.float32).reshape(B_PER_CORE, S, S)
        total[c * B_PER_CORE : (c + 1) * B_PER_CORE] = o
    return total[:, None, :, :]


if __name__ == "__main__":
    rng = np.random.default_rng(0)
    v = rng.random((S, S, S), dtype=np.float32)
    a = rng.standard_normal((B, 3, 3)).astype(np.float32)
    q, r = np.linalg.qr(a)
    rm = (q * np.sign(np.diagonal(r, axis1=-2, axis2=-1))[:, None, :]).astype(
        np.float32
    )
    out = kernel(rm, v, np.int64(3))
    print("out", out.shape, out.dtype, out.mean())
